# revision 3
# baseline (speedup 1.0000x reference)
"""Trainium2 Bass kernel for CachedLightningIndexer-style scoring.

Reference computation (b=2, t=s=4096, d_model=2048, heads=2, dim=32):
    q = (x @ wq).reshape(b, t, 2, 32); k = x @ wk; w = x @ ww
    scores[b,t,s] = sum_h w[b,t,h] * relu(q[b,t,h,:] . k[b,s,:])

Sharding (8 cores): output grid of 4 t-quarters x 2 s-halves over the
flattened (b*t) = 8192 rows. Core (i, j) computes scores for t rows
[2048*i, 2048*(i+1)) against s columns [2048*j, 2048*(j+1)) of batch
b = i // 2. Keys/queries/weights are all computed on-device from x;
the host only reshapes/transposes/casts inputs and concatenates outputs.

Per-core pipeline:
  phase 1: stream x^T d-tiles (bf16), project q|w (stationary wq|ww) and
           k (stationary wk) into PSUM with full-K accumulation.
  phase 2: per 128-row t-tile and 512-col s-chunk, two K=32 row-tiled
           fp32r matmuls (one per head) -> PSUM, then
           ACT: r0 = relu(d0), DVE: r1w = max(d1,0)*w1[t],
           GPSIMD: out = r0*w0[t] + r1w, DMA out.
"""

import numpy as np
import ml_dtypes

import concourse.bass as bass
import concourse.mybir as mybir
import concourse.tile as tile
from concourse import bacc
from concourse.bass_utils import run_bass_kernel_spmd
from concourse.masks import make_identity

BF16 = ml_dtypes.bfloat16

D_MODEL = 2048
HEADS = 2
DIM = 32
B = 2
T = 4096
N_CORES = 8
Q = 2048          # t rows per core
S = 2048          # s cols per core
KT = D_MODEL // 128   # 16 contraction tiles
NJ = Q // 128         # 16 t-tiles
NM = S // 512         # 4 s-chunks

OUT_BF16 = False      # f32 output by default

_cached = {}


def _build():
    out_dt = mybir.dt.bfloat16 if OUT_BF16 else mybir.dt.float32
    nc = bacc.Bacc("TRN2", target_bir_lowering=False, debug=False,
                   num_devices=N_CORES)
    xTt = nc.dram_tensor("xTt", [D_MODEL, Q], mybir.dt.bfloat16,
                         kind="ExternalInput").ap()
    xTs = nc.dram_tensor("xTs", [D_MODEL, S], mybir.dt.bfloat16,
                         kind="ExternalInput").ap()
    wqw = nc.dram_tensor("wqw", [D_MODEL, 66], mybir.dt.bfloat16,
                         kind="ExternalInput").ap()
    wkk = nc.dram_tensor("wkk", [D_MODEL, DIM], mybir.dt.bfloat16,
                         kind="ExternalInput").ap()
    out = nc.dram_tensor("out", [Q, S], out_dt, kind="ExternalOutput").ap()

    f32 = mybir.dt.float32
    bf16 = mybir.dt.bfloat16
    f32r = mybir.dt.float32r
    Alu = mybir.AluOpType
    Act = mybir.ActivationFunctionType

    with tile.TileContext(nc) as tc:
        with tc.tile_pool(name="wpool", bufs=1) as wpool, \
             tc.tile_pool(name="xpool", bufs=3) as xpool, \
             tc.tile_pool(name="spool", bufs=1) as spool:

            # stationary projection weights: [128, kt, cols]
            wqw_sb = wpool.tile([128, KT, 66], bf16)
            nc.sync.dma_start(wqw_sb[:], wqw.rearrange("(k p) c -> p k c", p=128))
            wk_sb = wpool.tile([128, KT, DIM], bf16)
            nc.sync.dma_start(wk_sb[:], wkk.rearrange("(k p) c -> p k c", p=128))
            ident2 = wpool.tile([2, 2], f32)
            make_identity(nc, ident2[:])

            qT = spool.tile([64, Q], f32r)        # (h,dim) x t
            kTd = spool.tile([64, S], f32r)       # duplicated keys^T (two 32-row copies)
            wT_sb = spool.tile([2, Q], f32)      # heads x t
            wvec = spool.tile([128, NJ, 2], f32)  # per-t-tile per-partition weights

            # ---- phase 1: projections with full-K accumulation ----
            with tc.tile_pool(name="psum1", bufs=1, space="PSUM") as ppool:
                ps_qw = ppool.tile([66, Q], f32)
                ps_k = ppool.tile([DIM, S], f32)
                for kt in range(KT):
                    xt = xpool.tile([128, Q], bf16, tag="xt")
                    nc.sync.dma_start(xt[:], xTt[kt * 128:(kt + 1) * 128, :])
                    xs = xpool.tile([128, S], bf16, tag="xs")
                    nc.sync.dma_start(xs[:], xTs[kt * 128:(kt + 1) * 128, :])
                    for n in range(NM):
                        sl = slice(n * 512, (n + 1) * 512)
                        nc.tensor.matmul(ps_qw[:, sl], lhsT=wqw_sb[:, kt, :],
                                         rhs=xt[:, sl],
                                         start=(kt == 0), stop=(kt == KT - 1))
                        nc.tensor.matmul(ps_k[:, sl], lhsT=wk_sb[:, kt, :],
                                         rhs=xs[:, sl],
                                         start=(kt == 0), stop=(kt == KT - 1))
                nc.scalar.copy(qT[:], ps_qw[0:64, :])
                nc.vector.tensor_copy(kTd[0:32, :], ps_k[:])
                nc.vector.tensor_copy(kTd[32:64, :], ps_k[:])
                nc.vector.tensor_copy(wT_sb[:], ps_qw[64:66, :])

            # ---- per-t-tile weight transposition: [2, 128] -> [128, 2] ----
            with tc.tile_pool(name="psumw", bufs=2, space="PSUM") as wtpool:
                for jj in range(NJ):
                    ps_w = wtpool.tile([128, 2], f32, tag="wt")
                    nc.tensor.transpose(ps_w[:], wT_sb[:, jj * 128:(jj + 1) * 128],
                                        ident2[:])
                    nc.scalar.copy(wvec[:, jj, :], ps_w[:])

            # ---- phase 2: dots + relu/weight/combine + store ----
            with tc.tile_pool(name="psumd", bufs=4, space="PSUM") as dpool, \
                 tc.tile_pool(name="rpool", bufs=3) as rpool, \
                 tc.tile_pool(name="opool", bufs=2) as opool:
                for jj in range(NJ):
                    tsl = slice(jj * 128, (jj + 1) * 128)
                    w0 = wvec[:, jj, 0:1]
                    w1 = wvec[:, jj, 1:2]
                    ot = opool.tile([128, S], out_dt, tag="ot")
                    for m in range(NM):
                        ssl = slice(m * 512, (m + 1) * 512)
                        d0 = dpool.tile([128, 512], f32, tag="d")
                        d1 = dpool.tile([128, 512], f32, tag="d")
                        nc.tensor.matmul(d0[:], lhsT=qT[0:32, tsl],
                                         rhs=kTd[0:32, ssl],
                                         tile_position=(0, 0))
                        nc.tensor.matmul(d1[:], lhsT=qT[32:64, tsl],
                                         rhs=kTd[32:64, ssl],
                                         tile_position=(32, 0))
                        r0 = rpool.tile([128, 512], bf16, tag="r0")
                        nc.scalar.activation(r0[:], d0[:], Act.Relu)
                        r1 = rpool.tile([128, 512], bf16, tag="r1")
                        nc.vector.tensor_scalar(r1[:], d1[:], 0.0, w1,
                                                Alu.max, Alu.mult)
                        nc.vector.scalar_tensor_tensor(ot[:, ssl], r0[:], w0, r1[:],
                                                       Alu.mult, Alu.add)
                    nc.sync.dma_start(out[tsl, :], ot[:])
    nc.compile()
    return nc


def _get_nc():
    if "nc" not in _cached:
        _cached["nc"] = _build()
    return _cached["nc"]


def _make_in_maps(x, wq, wk, ww):
    x_flat = np.asarray(x, dtype=np.float32).reshape(B * T, D_MODEL)
    xT = x_flat.T.astype(BF16)          # [2048, 8192], C-contiguous
    wqw = np.concatenate([np.asarray(wq), np.asarray(ww)], axis=1).astype(BF16)
    wkk = np.asarray(wk).astype(BF16)
    in_maps = []
    for core in range(N_CORES):
        i, j = core // 2, core % 2
        b = i // 2
        t_lo = i * Q
        s_lo = b * T + j * S
        in_maps.append({
            "xTt": np.ascontiguousarray(xT[:, t_lo:t_lo + Q]),
            "xTs": np.ascontiguousarray(xT[:, s_lo:s_lo + S]),
            "wqw": wqw,
            "wkk": wkk,
        })
    return in_maps


def run(x, wq, wk, ww, trace=False, **kw):
    nc = _get_nc()
    in_maps = _make_in_maps(x, wq, wk, ww)
    res = run_bass_kernel_spmd(nc, in_maps, list(range(N_CORES)),
                               trace=trace, **kw)
    out = np.empty((B * T, T), dtype=np.float32)
    for core in range(N_CORES):
        i, j = core // 2, core % 2
        blk = res.results[core]["out"]
        out[i * Q:(i + 1) * Q, j * S:(j + 1) * S] = blk.astype(np.float32)
    return out.reshape(B, T, T), res


def kernel(x, wq, wk, ww):
    out, _ = run(x, wq, wk, ww, trace=False)
    return out


# revision 4
# speedup vs baseline: 1.0046x; 1.0046x over previous
"""Trainium2 Bass kernel for CachedLightningIndexer-style scoring.

Reference computation (b=2, t=s=4096, d_model=2048, heads=2, dim=32):
    q = (x @ wq).reshape(b, t, 2, 32); k = x @ wk; w = x @ ww
    scores[b,t,s] = sum_h w[b,t,h] * relu(q[b,t,h,:] . k[b,s,:])

Sharding (8 cores): output grid of 4 t-quarters x 2 s-halves over the
flattened (b*t) = 8192 rows. Core (i, j) computes scores for t rows
[2048*i, 2048*(i+1)) against s columns [2048*j, 2048*(j+1)) of batch
b = i // 2. Keys/queries/weights are all computed on-device from x;
the host only reshapes/transposes/casts inputs and concatenates outputs.

Per-core pipeline:
  phase 1: stream x^T d-tiles (bf16), project q|w (stationary wq|ww) and
           k (stationary wk) into PSUM with full-K accumulation.
  phase 2: per 128-row t-tile and 512-col s-chunk, two K=32 row-tiled
           fp32r matmuls (one per head) -> PSUM, then
           ACT: r0 = relu(d0), DVE: r1w = max(d1,0)*w1[t],
           GPSIMD: out = r0*w0[t] + r1w, DMA out.
"""

import numpy as np
import ml_dtypes

import concourse.bass as bass
import concourse.mybir as mybir
import concourse.tile as tile
from concourse import bacc
from concourse.bass_utils import run_bass_kernel_spmd
from concourse.masks import make_identity

BF16 = ml_dtypes.bfloat16

D_MODEL = 2048
HEADS = 2
DIM = 32
B = 2
T = 4096
N_CORES = 8
Q = 2048          # t rows per core
S = 2048          # s cols per core
KT = D_MODEL // 128   # 16 contraction tiles
NJ = Q // 128         # 16 t-tiles
NM = S // 512         # 4 s-chunks

OUT_BF16 = True       # bf16 output, host upcasts (err ~5e-3 vs 2e-2 gate)

_cached = {}


def _build():
    out_dt = mybir.dt.bfloat16 if OUT_BF16 else mybir.dt.float32
    nc = bacc.Bacc("TRN2", target_bir_lowering=False, debug=False,
                   num_devices=N_CORES)
    xTt = nc.dram_tensor("xTt", [D_MODEL, Q], mybir.dt.bfloat16,
                         kind="ExternalInput").ap()
    xTs = nc.dram_tensor("xTs", [D_MODEL, S], mybir.dt.bfloat16,
                         kind="ExternalInput").ap()
    wqw = nc.dram_tensor("wqw", [D_MODEL, 66], mybir.dt.bfloat16,
                         kind="ExternalInput").ap()
    wkk = nc.dram_tensor("wkk", [D_MODEL, DIM], mybir.dt.bfloat16,
                         kind="ExternalInput").ap()
    out = nc.dram_tensor("out", [Q, S], out_dt, kind="ExternalOutput").ap()

    f32 = mybir.dt.float32
    bf16 = mybir.dt.bfloat16
    f32r = mybir.dt.float32r
    Alu = mybir.AluOpType
    Act = mybir.ActivationFunctionType

    with tile.TileContext(nc) as tc:
        with tc.tile_pool(name="wpool", bufs=1) as wpool, \
             tc.tile_pool(name="xpool", bufs=4) as xpool, \
             tc.tile_pool(name="spool", bufs=1) as spool:

            # stationary projection weights: [128, kt, cols]
            wqw_sb = wpool.tile([128, KT, 66], bf16)
            nc.sync.dma_start(wqw_sb[:], wqw.rearrange("(k p) c -> p k c", p=128))
            wk_sb = wpool.tile([128, KT, DIM], bf16)
            nc.sync.dma_start(wk_sb[:], wkk.rearrange("(k p) c -> p k c", p=128))
            ident2 = wpool.tile([2, 2], f32)
            make_identity(nc, ident2[:])

            qT = spool.tile([64, Q], bf16)        # (h,dim) x t
            kTd = spool.tile([64, S], bf16)       # duplicated keys^T (two 32-row copies)
            wT_sb = spool.tile([2, Q], f32)      # heads x t
            wvec = spool.tile([128, NJ, 2], f32)  # per-t-tile per-partition weights

            # ---- phase 1: projections with full-K accumulation ----
            with tc.tile_pool(name="psum1", bufs=1, space="PSUM") as ppool:
                ps_qw = ppool.tile([66, Q], f32)
                ps_k = ppool.tile([DIM, S], f32)
                for kt in range(KT):
                    xt = xpool.tile([128, Q], bf16, tag="xt")
                    nc.sync.dma_start(xt[:], xTt[kt * 128:(kt + 1) * 128, :])
                    xs = xpool.tile([128, S], bf16, tag="xs")
                    nc.sync.dma_start(xs[:], xTs[kt * 128:(kt + 1) * 128, :])
                    for n in range(NM):
                        sl = slice(n * 512, (n + 1) * 512)
                        nc.tensor.matmul(ps_qw[:, sl], lhsT=wqw_sb[:, kt, :],
                                         rhs=xt[:, sl],
                                         start=(kt == 0), stop=(kt == KT - 1))
                        nc.tensor.matmul(ps_k[:, sl], lhsT=wk_sb[:, kt, :],
                                         rhs=xs[:, sl],
                                         start=(kt == 0), stop=(kt == KT - 1))
                nc.scalar.copy(qT[:], ps_qw[0:64, :])
                nc.vector.tensor_copy(kTd[0:32, :], ps_k[:])
                nc.vector.tensor_copy(kTd[32:64, :], ps_k[:])
                nc.vector.tensor_copy(wT_sb[:], ps_qw[64:66, :])

            # ---- per-t-tile weight transposition: [2, 128] -> [128, 2] ----
            with tc.tile_pool(name="psumw", bufs=2, space="PSUM") as wtpool:
                for jj in range(NJ):
                    ps_w = wtpool.tile([128, 2], f32, tag="wt")
                    nc.tensor.transpose(ps_w[:], wT_sb[:, jj * 128:(jj + 1) * 128],
                                        ident2[:])
                    nc.scalar.copy(wvec[:, jj, :], ps_w[:])

            # ---- phase 2: dots + relu/weight/combine + store ----
            with tc.tile_pool(name="psumd", bufs=6, space="PSUM") as dpool, \
                 tc.tile_pool(name="rpool", bufs=4) as rpool, \
                 tc.tile_pool(name="opool", bufs=2) as opool:
                for jj in range(NJ):
                    tsl = slice(jj * 128, (jj + 1) * 128)
                    w0 = wvec[:, jj, 0:1]
                    w1 = wvec[:, jj, 1:2]
                    ot = opool.tile([128, S], out_dt, tag="ot")
                    for m in range(NM):
                        ssl = slice(m * 512, (m + 1) * 512)
                        d0 = dpool.tile([128, 512], f32, tag="d")
                        d1 = dpool.tile([128, 512], f32, tag="d")
                        nc.tensor.matmul(d0[:], lhsT=qT[0:32, tsl],
                                         rhs=kTd[0:32, ssl],
                                         tile_position=(0, 0))
                        nc.tensor.matmul(d1[:], lhsT=qT[32:64, tsl],
                                         rhs=kTd[32:64, ssl],
                                         tile_position=(32, 0))
                        r0 = rpool.tile([128, 512], bf16, tag="r0")
                        nc.scalar.activation(r0[:], d0[:], Act.Relu)
                        r1 = rpool.tile([128, 512], bf16, tag="r1")
                        nc.vector.tensor_scalar(r1[:], d1[:], 0.0, w1,
                                                Alu.max, Alu.mult)
                        nc.vector.scalar_tensor_tensor(ot[:, ssl], r0[:], w0, r1[:],
                                                       Alu.mult, Alu.add)
                    nc.sync.dma_start(out[tsl, :], ot[:])
    nc.compile()
    return nc


def _get_nc():
    if "nc" not in _cached:
        _cached["nc"] = _build()
    return _cached["nc"]


def _make_in_maps(x, wq, wk, ww):
    x_flat = np.asarray(x, dtype=np.float32).reshape(B * T, D_MODEL)
    xT = x_flat.T.astype(BF16)          # [2048, 8192], C-contiguous
    wqw = np.concatenate([np.asarray(wq), np.asarray(ww)], axis=1).astype(BF16)
    wkk = np.asarray(wk).astype(BF16)
    in_maps = []
    for core in range(N_CORES):
        i, j = core // 2, core % 2
        b = i // 2
        t_lo = i * Q
        s_lo = b * T + j * S
        in_maps.append({
            "xTt": np.ascontiguousarray(xT[:, t_lo:t_lo + Q]),
            "xTs": np.ascontiguousarray(xT[:, s_lo:s_lo + S]),
            "wqw": wqw,
            "wkk": wkk,
        })
    return in_maps


def run(x, wq, wk, ww, trace=False, **kw):
    nc = _get_nc()
    in_maps = _make_in_maps(x, wq, wk, ww)
    res = run_bass_kernel_spmd(nc, in_maps, list(range(N_CORES)),
                               trace=trace, **kw)
    out = np.empty((B * T, T), dtype=np.float32)
    for core in range(N_CORES):
        i, j = core // 2, core % 2
        blk = res.results[core]["out"]
        out[i * Q:(i + 1) * Q, j * S:(j + 1) * S] = blk.astype(np.float32)
    return out.reshape(B, T, T), res


def kernel(x, wq, wk, ww):
    out, _ = run(x, wq, wk, ww, trace=False)
    return out


# revision 5
# speedup vs baseline: 1.1019x; 1.0969x over previous
"""Trainium2 Bass kernel for CachedLightningIndexer-style scoring.

Reference computation (b=2, t=s=4096, d_model=2048, heads=2, dim=32):
    q = (x @ wq).reshape(b, t, 2, 32); k = x @ wk; w = x @ ww
    scores[b,t,s] = sum_h w[b,t,h] * relu(q[b,t,h,:] . k[b,s,:])

Sharding (8 cores): output grid of 4 t-quarters x 2 s-halves over the
flattened (b*t) = 8192 rows. Core (i, j) computes scores for t rows
[2048*i, 2048*(i+1)) against s columns [2048*j, 2048*(j+1)) of batch
b = i // 2. Keys/queries/weights are all computed on-device from x;
the host only reshapes/transposes/casts inputs and concatenates outputs.

Per-core pipeline (s pipelined in 2 chunks of 1024 to overlap input DMA
with compute):
  phase 1: stream x^T d-tiles (bf16); project q|w (stationary wq|ww over
           the full t slab) and k per s-chunk, full-K PSUM accumulation.
  phase 2 (per s-chunk): per 128-row t-tile, two K=32 matmuls per
           512-col half -> d0/d1 [128,1024] PSUM, then with
           r = w1/w0 (per-partition):
      plan A: ACT r0=relu(d0); DVE r1=(max(d1,0))*r; DVE v=r0+r1;
              DVE out=v*w0
      plan B: ACT r0=relu(d0); ACT r1u=relu(d1); DVE r1=r1u*r;
              DVE v=r0+r1; DVE out=v*w0
  (plans mixed ~55/45 across tiles to balance ACT vs DVE load)
"""

import numpy as np
import ml_dtypes

import concourse.bass as bass
import concourse.mybir as mybir
import concourse.tile as tile
from concourse import bacc
from concourse.bass_utils import run_bass_kernel_spmd
from concourse.masks import make_identity

BF16 = ml_dtypes.bfloat16

D_MODEL = 2048
HEADS = 2
DIM = 32
B = 2
T = 4096
N_CORES = 8
Q = 2048              # t rows per core
S = 2048              # s cols per core
KT = D_MODEL // 128   # 16 contraction tiles
NJ = Q // 128         # 16 t-tiles
NC = 2                # s-chunks
SC = S // NC          # 1024 cols per chunk

_cached = {}


def _build():
    out_dt = mybir.dt.bfloat16
    nc = bacc.Bacc("TRN2", target_bir_lowering=False, debug=False,
                   num_devices=N_CORES)
    xTt = nc.dram_tensor("xTt", [D_MODEL, Q], mybir.dt.bfloat16,
                         kind="ExternalInput").ap()
    xTs = nc.dram_tensor("xTs", [D_MODEL, S], mybir.dt.bfloat16,
                         kind="ExternalInput").ap()
    wqw = nc.dram_tensor("wqw", [D_MODEL, 66], mybir.dt.bfloat16,
                         kind="ExternalInput").ap()
    wkk = nc.dram_tensor("wkk", [D_MODEL, DIM], mybir.dt.bfloat16,
                         kind="ExternalInput").ap()
    out = nc.dram_tensor("out", [Q, S], out_dt, kind="ExternalOutput").ap()

    f32 = mybir.dt.float32
    bf16 = mybir.dt.bfloat16
    Alu = mybir.AluOpType
    Act = mybir.ActivationFunctionType

    with tile.TileContext(nc) as tc:
        with tc.tile_pool(name="wpool", bufs=1) as wpool, \
             tc.tile_pool(name="xpool", bufs=4) as xpool, \
             tc.tile_pool(name="spool", bufs=1) as spool:

            wqw_sb = wpool.tile([128, KT, 66], bf16)
            nc.sync.dma_start(wqw_sb[:], wqw.rearrange("(k p) c -> p k c", p=128))
            wk_sb = wpool.tile([128, KT, DIM], bf16)
            nc.sync.dma_start(wk_sb[:], wkk.rearrange("(k p) c -> p k c", p=128))
            ident2 = wpool.tile([2, 2], f32)
            make_identity(nc, ident2[:])

            qT = spool.tile([64, Q], bf16)       # (h,dim) x t
            kTd = spool.tile([64, S], bf16)      # duplicated keys^T
            wT_sb = spool.tile([2, Q], f32)      # heads x t
            wvec = spool.tile([128, NJ, 2], f32)
            rvec = spool.tile([128, NJ], f32)    # w1/w0, clamped

            # ---- phase 1: projections ----
            with tc.tile_pool(name="psum1", bufs=1, space="PSUM") as ppool:
                ps_qw = ppool.tile([66, Q], f32)
                ps_k = [ppool.tile([DIM, SC], f32, name=f"ps_k{c}") for c in range(NC)]
                for kt in range(KT):
                    xt = xpool.tile([128, Q], bf16, tag="xt")
                    nc.sync.dma_start(xt[:], xTt[kt * 128:(kt + 1) * 128, :])
                    for n in range(Q // 512):
                        sl = slice(n * 512, (n + 1) * 512)
                        nc.tensor.matmul(ps_qw[:, sl], lhsT=wqw_sb[:, kt, :],
                                         rhs=xt[:, sl],
                                         start=(kt == 0), stop=(kt == KT - 1))
                # chunk 0 keys right away; chunk 1 streams later
                for c in range(NC):
                    csl = slice(c * SC, (c + 1) * SC)
                    for kt in range(KT):
                        xs = xpool.tile([128, SC], bf16, tag="xs")
                        nc.sync.dma_start(xs[:], xTs[kt * 128:(kt + 1) * 128, csl])
                        for n in range(SC // 512):
                            sl = slice(n * 512, (n + 1) * 512)
                            nc.tensor.matmul(ps_k[c][:, sl], lhsT=wk_sb[:, kt, :],
                                             rhs=xs[:, sl],
                                             start=(kt == 0), stop=(kt == KT - 1))
                # copies out of PSUM (qT/wT as soon as qw-proj is done)
                nc.scalar.copy(qT[:], ps_qw[0:64, :])
                nc.vector.tensor_copy(wT_sb[:], ps_qw[64:66, :])
                for c in range(NC):
                    csl = slice(c * SC, (c + 1) * SC)
                    nc.vector.tensor_copy(kTd[0:32, csl], ps_k[c][:])
                    nc.vector.tensor_copy(kTd[32:64, csl], ps_k[c][:])

            # ---- per-t-tile weights: transpose + ratio ----
            with tc.tile_pool(name="psumw", bufs=2, space="PSUM") as wtpool:
                for jj in range(NJ):
                    ps_w = wtpool.tile([128, 2], f32, tag="wt")
                    nc.tensor.transpose(ps_w[:], wT_sb[:, jj * 128:(jj + 1) * 128],
                                        ident2[:])
                    nc.scalar.copy(wvec[:, jj, :], ps_w[:])
            w0recip = spool.tile([128, NJ], f32)
            nc.vector.reciprocal(w0recip[:], wvec[:, :, 0])
            nc.vector.tensor_tensor(rvec[:], wvec[:, :, 1], w0recip[:], Alu.mult)
            nc.vector.tensor_scalar(rvec[:], rvec[:], 1e20, -1e20, Alu.min, Alu.max)

            # ---- phase 2 ----
            with tc.tile_pool(name="psumd", bufs=3, space="PSUM") as dpool, \
                 tc.tile_pool(name="rpool", bufs=4) as rpool, \
                 tc.tile_pool(name="opool", bufs=4) as opool:
                pos = 0
                for c in range(NC):
                    csl = slice(c * SC, (c + 1) * SC)
                    for jj in range(NJ):
                        tsl = slice(jj * 128, (jj + 1) * 128)
                        w0 = wvec[:, jj, 0:1]
                        rv = rvec[:, jj:jj + 1]
                        plan_b = (pos * 4) % 9 < 4   # ~44% plan B
                        pos += 1
                        d0 = dpool.tile([128, SC], f32, tag="d")
                        d1 = dpool.tile([128, SC], f32, tag="d")
                        for n in range(SC // 512):
                            sl = slice(n * 512, (n + 1) * 512)
                            ksl = slice(c * SC + n * 512, c * SC + (n + 1) * 512)
                            nc.tensor.matmul(d0[:, sl], lhsT=qT[0:32, tsl],
                                             rhs=kTd[0:32, ksl],
                                             tile_position=(0, 0))
                            nc.tensor.matmul(d1[:, sl], lhsT=qT[32:64, tsl],
                                             rhs=kTd[32:64, ksl],
                                             tile_position=(32, 0))
                        r0 = rpool.tile([128, SC], bf16, tag="r0")
                        nc.scalar.activation(r0[:], d0[:], Act.Relu)
                        r1 = rpool.tile([128, SC], bf16, tag="r1")
                        if plan_b:
                            r1u = rpool.tile([128, SC], bf16, tag="r1u")
                            nc.scalar.activation(r1u[:], d1[:], Act.Relu)
                            nc.vector.tensor_scalar(r1[:], r1u[:], rv, None,
                                                    Alu.mult)
                        else:
                            nc.vector.tensor_scalar(r1[:], d1[:], 0.0, rv,
                                                    Alu.max, Alu.mult)
                        v = rpool.tile([128, SC], bf16, tag="v")
                        nc.vector.tensor_tensor(v[:], r0[:], r1[:], Alu.add)
                        ot = opool.tile([128, SC], out_dt, tag="ot")
                        nc.vector.tensor_scalar(ot[:], v[:], w0, None, Alu.mult)
                        nc.sync.dma_start(out[tsl, csl], ot[:])
    nc.compile()
    return nc


def _get_nc():
    if "nc" not in _cached:
        _cached["nc"] = _build()
    return _cached["nc"]


def _make_in_maps(x, wq, wk, ww):
    x_flat = np.asarray(x, dtype=np.float32).reshape(B * T, D_MODEL)
    xT = x_flat.T.astype(BF16)          # [2048, 8192], C-contiguous
    wqw = np.concatenate([np.asarray(wq), np.asarray(ww)], axis=1).astype(BF16)
    wkk = np.asarray(wk).astype(BF16)
    in_maps = []
    for core in range(N_CORES):
        i, j = core // 2, core % 2
        b = i // 2
        t_lo = i * Q
        s_lo = b * T + j * S
        in_maps.append({
            "xTt": np.ascontiguousarray(xT[:, t_lo:t_lo + Q]),
            "xTs": np.ascontiguousarray(xT[:, s_lo:s_lo + S]),
            "wqw": wqw,
            "wkk": wkk,
        })
    return in_maps


def run(x, wq, wk, ww, trace=False, **kw):
    nc = _get_nc()
    in_maps = _make_in_maps(x, wq, wk, ww)
    res = run_bass_kernel_spmd(nc, in_maps, list(range(N_CORES)),
                               trace=trace, **kw)
    out = np.empty((B * T, T), dtype=np.float32)
    for core in range(N_CORES):
        i, j = core // 2, core % 2
        blk = res.results[core]["out"]
        out[i * Q:(i + 1) * Q, j * S:(j + 1) * S] = blk.astype(np.float32)
    return out.reshape(B, T, T), res


def kernel(x, wq, wk, ww):
    out, _ = run(x, wq, wk, ww, trace=False)
    return out


# revision 7
# speedup vs baseline: 1.2304x; 1.1166x over previous
"""Trainium2 Bass kernel for CachedLightningIndexer-style scoring.

Reference computation (b=2, t=s=4096, d_model=2048, heads=2, dim=32):
    q = (x @ wq).reshape(b, t, 2, 32); k = x @ wk; w = x @ ww
    scores[b,t,s] = sum_h w[b,t,h] * relu(q[b,t,h,:] . k[b,s,:])

Sharding (8 cores): output grid of 4 t-quarters x 2 s-halves over the
flattened (b*t) = 8192 rows. Core (i, j) computes scores for t rows
[2048*i, 2048*(i+1)) against s columns [2048*j, 2048*(j+1)) of batch
b = i // 2. Everything is computed on-device from x; the host only
reshapes/transposes/casts inputs and concatenates outputs.

Per-core pipeline (s pipelined in 2 chunks of 1024):
  phase 1: big resident loads of x^T (bf16, [128, kt, cols] layout);
           project q|w over the full t slab and k per s-chunk with
           full-K PSUM accumulation. Chunk-1 key tiles use a small
           pool so their DMAs trail behind chunk-0 compute.
  phase 2 (per s-chunk, per 128-row t-tile): two K=32 matmuls per
           512-col half -> d0/d1 [128,1024] PSUM; with per-partition
           r = w1/w0 (clamped):
      plan A: ACT r0=relu(d0); DVE r1=max(d1,0)*r; v=r0+r1; out=v*w0
      plan B: ACT r0=relu(d0), r1u=relu(d1); DVE r1=r1u*r; v=r0+r1;
              out=v*w0
  (plan B on ~60% of tiles balances ACT vs DVE)
"""

import numpy as np
import ml_dtypes

import concourse.bass as bass
import concourse.mybir as mybir
import concourse.tile as tile
from concourse import bacc
from concourse.bass_utils import run_bass_kernel_spmd
from concourse.masks import make_identity

BF16 = ml_dtypes.bfloat16

D_MODEL = 2048
B = 2
T = 4096
DIM = 32
N_CORES = 8
Q = 2048
S = 2048
KT = D_MODEL // 128   # 16
NJ = Q // 128         # 16
NC = 2
SC = S // NC          # 1024

PLAN_B_FRAC = 0.6

_cached = {}


def _build():
    out_dt = mybir.dt.bfloat16
    nc = bacc.Bacc("TRN2", target_bir_lowering=False, debug=False,
                   num_devices=N_CORES)
    xTt = nc.dram_tensor("xTt", [128, KT, Q], mybir.dt.bfloat16,
                         kind="ExternalInput").ap()
    xTs = nc.dram_tensor("xTs", [128, KT, S], mybir.dt.bfloat16,
                         kind="ExternalInput").ap()
    wqw = nc.dram_tensor("wqw", [D_MODEL, 66], mybir.dt.bfloat16,
                         kind="ExternalInput").ap()
    wkk = nc.dram_tensor("wkk", [D_MODEL, DIM], mybir.dt.bfloat16,
                         kind="ExternalInput").ap()
    out = nc.dram_tensor("out", [Q, S], out_dt, kind="ExternalOutput").ap()

    f32 = mybir.dt.float32
    bf16 = mybir.dt.bfloat16
    Alu = mybir.AluOpType
    Act = mybir.ActivationFunctionType

    with tile.TileContext(nc) as tc:
        with tc.tile_pool(name="wpool", bufs=1) as wpool, \
             tc.tile_pool(name="xpool", bufs=1) as xpool, \
             tc.tile_pool(name="xs0pool", bufs=KT) as xs0pool, \
             tc.tile_pool(name="xs1pool", bufs=3) as xs1pool, \
             tc.tile_pool(name="spool", bufs=1) as spool:

            wqw_sb = wpool.tile([128, KT, 66], bf16)
            nc.sync.dma_start(wqw_sb[:], wqw.rearrange("(k p) c -> p k c", p=128))
            wk_sb = wpool.tile([128, KT, DIM], bf16)
            nc.sync.dma_start(wk_sb[:], wkk.rearrange("(k p) c -> p k c", p=128))
            ident2 = wpool.tile([2, 2], f32)
            make_identity(nc, ident2[:])

            # resident x^T slab for queries/weights: 4 big DMAs
            xt_all = xpool.tile([128, KT, Q], bf16)
            for i in range(4):
                nc.sync.dma_start(xt_all[:, i * 4:(i + 1) * 4, :],
                                  xTt[:, i * 4:(i + 1) * 4, :])
            # chunk-0 keys: per-kt tiles, all slots resident -> DMAs run freely
            xs0 = []
            for kt in range(KT):
                t_ = xs0pool.tile([128, SC], bf16, tag="xs0", name=f"xs0_{kt}")
                nc.sync.dma_start(t_[:], xTs[:, kt, 0:SC])
                xs0.append(t_)

            qT0 = spool.tile([32, Q], bf16)
            qT1 = spool.tile([32, Q], bf16)
            kT = spool.tile([32, S], bf16)
            wT_sb = spool.tile([2, Q], f32)
            wvec = spool.tile([128, NJ, 2], f32)
            rvec = spool.tile([128, NJ], f32)

            with tc.tile_pool(name="psK1", bufs=1, space="PSUM") as psK1:
                ps_k1 = psK1.tile([DIM, SC], f32)
                with tc.tile_pool(name="psK0", bufs=1, space="PSUM") as psK0:
                    ps_k0 = psK0.tile([DIM, SC], f32)
                    with tc.tile_pool(name="psA", bufs=1, space="PSUM") as psA:
                        ps_qw = psA.tile([66, SC], f32)
                        # qw projection, two sequential t-halves (small PSUM)
                        for h in range(2):
                            hsl = slice(h * SC, (h + 1) * SC)
                            for kt in range(KT):
                                st, sp = (kt == 0), (kt == KT - 1)
                                for n in range(SC // 512):
                                    sl = slice(n * 512, (n + 1) * 512)
                                    gsl = slice(h * SC + n * 512,
                                                h * SC + (n + 1) * 512)
                                    nc.tensor.matmul(
                                        ps_qw[:, sl], lhsT=wqw_sb[:, kt, :],
                                        rhs=xt_all[:, kt, gsl], start=st, stop=sp)
                            nc.scalar.copy(qT0[:, hsl], ps_qw[0:32, :])
                            nc.scalar.copy(qT1[:, hsl], ps_qw[32:64, :])
                            nc.vector.tensor_copy(wT_sb[:, hsl], ps_qw[64:66, :])
                        for kt in range(KT):
                            st, sp = (kt == 0), (kt == KT - 1)
                            for n in range(SC // 512):
                                sl = slice(n * 512, (n + 1) * 512)
                                nc.tensor.matmul(ps_k0[:, sl], lhsT=wk_sb[:, kt, :],
                                                 rhs=xs0[kt][:, sl],
                                                 start=st, stop=sp)
                        nc.vector.tensor_copy(kT[:, 0:SC], ps_k0[:])

                    # weight transposes + ratio prep (psA closed: 2 banks free)
                    with tc.tile_pool(name="psumw", bufs=2, space="PSUM") as wtpool:
                        for jj in range(NJ):
                            ps_w = wtpool.tile([128, 2], f32, tag="wt")
                            nc.tensor.transpose(
                                ps_w[:], wT_sb[:, jj * 128:(jj + 1) * 128],
                                ident2[:])
                            nc.scalar.copy(wvec[:, jj, :], ps_w[:])
                    w0recip = spool.tile([128, NJ], f32)
                    nc.vector.reciprocal(w0recip[:], wvec[:, :, 0])
                    nc.vector.tensor_tensor(rvec[:], wvec[:, :, 1], w0recip[:],
                                            Alu.mult)
                    nc.vector.tensor_scalar(rvec[:], rvec[:], 1e20, -1e20,
                                            Alu.min, Alu.max)

                # ---- phase 2 (psA/psK0/psumw closed: 6 banks free) ----
                with tc.tile_pool(name="psumd", bufs=3, space="PSUM") as dpool, \
                     tc.tile_pool(name="rpool", bufs=4) as rpool, \
                     tc.tile_pool(name="opool", bufs=4) as opool:

                    def do_chunk(c, pos0):
                        csl = slice(c * SC, (c + 1) * SC)
                        for jj in range(NJ):
                            pos = pos0 + jj
                            tsl = slice(jj * 128, (jj + 1) * 128)
                            w0 = wvec[:, jj, 0:1]
                            rv = rvec[:, jj:jj + 1]
                            plan_b = (pos * 3) % 5 < 3   # 60% plan B
                            d0 = dpool.tile([128, SC], f32, tag="d")
                            d1 = dpool.tile([128, SC], f32, tag="d")
                            for n in range(SC // 512):
                                sl = slice(n * 512, (n + 1) * 512)
                                ksl = slice(c * SC + n * 512, c * SC + (n + 1) * 512)
                                nc.tensor.matmul(d0[:, sl], lhsT=qT0[:, tsl],
                                                 rhs=kT[:, ksl])
                                nc.tensor.matmul(d1[:, sl], lhsT=qT1[:, tsl],
                                                 rhs=kT[:, ksl])
                            r0 = rpool.tile([128, SC], bf16, tag="r0")
                            nc.scalar.activation(r0[:], d0[:], Act.Relu)
                            r1 = rpool.tile([128, SC], bf16, tag="r1")
                            if plan_b:
                                r1u = rpool.tile([128, SC], bf16, tag="r1u")
                                nc.scalar.activation(r1u[:], d1[:], Act.Relu)
                                nc.vector.tensor_scalar(r1[:], r1u[:], rv, None,
                                                        Alu.mult)
                            else:
                                nc.vector.tensor_scalar(r1[:], d1[:], 0.0, rv,
                                                        Alu.max, Alu.mult)
                            v = rpool.tile([128, SC], bf16, tag="v")
                            nc.vector.tensor_tensor(v[:], r0[:], r1[:], Alu.add)
                            ot = opool.tile([128, SC], out_dt, tag="ot")
                            nc.vector.tensor_scalar(ot[:], v[:], w0, None, Alu.mult)
                            nc.sync.dma_start(out[tsl, csl], ot[:])

                    do_chunk(0, 0)

                    # chunk-1 keys: loads trail chunk-0 compute via small pool
                    for kt in range(KT):
                        xs = xs1pool.tile([128, SC], bf16, tag="xs1")
                        nc.sync.dma_start(xs[:], xTs[:, kt, SC:2 * SC])
                        st, sp = (kt == 0), (kt == KT - 1)
                        for n in range(SC // 512):
                            sl = slice(n * 512, (n + 1) * 512)
                            nc.tensor.matmul(ps_k1[:, sl], lhsT=wk_sb[:, kt, :],
                                             rhs=xs[:, sl], start=st, stop=sp)
                    nc.vector.tensor_copy(kT[:, SC:2 * SC], ps_k1[:])

                    do_chunk(1, NJ)
    nc.compile()
    return nc


def _get_nc():
    if "nc" not in _cached:
        _cached["nc"] = _build()
    return _cached["nc"]


def _make_in_maps(x, wq, wk, ww):
    x_flat = np.asarray(x, dtype=np.float32).reshape(B * T, D_MODEL)
    xT = x_flat.T.astype(BF16)                       # [2048, 8192]
    xTr = np.ascontiguousarray(                      # [128, 16, 8192]
        xT.reshape(KT, 128, B * T).transpose(1, 0, 2))
    wqw = np.concatenate([np.asarray(wq), np.asarray(ww)], axis=1).astype(BF16)
    wkk = np.asarray(wk).astype(BF16)
    in_maps = []
    for core in range(N_CORES):
        i, j = core // 2, core % 2
        b = i // 2
        t_lo = i * Q
        s_lo = b * T + j * S
        in_maps.append({
            "xTt": np.ascontiguousarray(xTr[:, :, t_lo:t_lo + Q]),
            "xTs": np.ascontiguousarray(xTr[:, :, s_lo:s_lo + S]),
            "wqw": wqw,
            "wkk": wkk,
        })
    return in_maps


def run(x, wq, wk, ww, trace=False, **kw):
    nc = _get_nc()
    in_maps = _make_in_maps(x, wq, wk, ww)
    res = run_bass_kernel_spmd(nc, in_maps, list(range(N_CORES)),
                               trace=trace, **kw)
    out = np.empty((B * T, T), dtype=np.float32)
    for core in range(N_CORES):
        i, j = core // 2, core % 2
        blk = res.results[core]["out"]
        out[i * Q:(i + 1) * Q, j * S:(j + 1) * S] = blk.astype(np.float32)
    return out.reshape(B, T, T), res


def kernel(x, wq, wk, ww):
    out, _ = run(x, wq, wk, ww, trace=False)
    return out
